# revision 2
# baseline (speedup 1.0000x reference)
"""Trainium2 Bass kernel for nn_DCTHighPass.

Reference computation (per sample, 512x512 RGB image):
  gray = 0.299 R + 0.587 G + 0.114 B
  tiles = 8x8 blocks of gray (64x64 tiles, row-major (ti, tj))
  mag = |fft2(tile)|
  (buggy mask touches only batch 3:6 / fft rows 3:6 -> never sampled below)
  img = mag tiles stacked into [4096*8, 8]
  out = bilinear_resize(img, 512, 512)

Key reduction: the height downsample (32768 -> 512, factor 64) samples only
input rows 64i+31 and 64i+32 with weight 0.5 each, i.e. fft-row 7 of tile
(ti=i//8, tj=8*(i%8)+3) and fft-row 0 of tile (ti, tj=8*(i%8)+4).  fft rows
0/7 of an 8x8 real tile need only three 8-weight row reductions of the tile
(plain sum, cos, sin), followed by an 8-point DFT along columns.  The width
upsample (8 -> 512) is a fixed [8,512] matrix.  So per output row i:
  v = 0.5*(|DFT(cos/sin rowsums of tile tj3)| + |DFT(colsum of tile tj4)|)
  out[i, :] = v @ W8
Only image columns 64p+24 .. 64p+39 (p = i%8) are ever used, so the host
pre-gathers exactly those 128 columns per row into a [B, H, 3, 128]
row-major tensor; every DMA run is then >= 1536B contiguous (full DMA-bus
rate; the original 64B gather runs paid the sub-512B 2x latency penalty).

Pipeline per sample (per core, batch of 8 samples):
  load  (1 DMA):  xs[smp] -> SBUF [128=(row%128), (q, ch, c)=1536]
  gray  (ACT/DVE): 0.299 R + 0.587 G + 0.114 B -> [128, (q c)=512]
  stage1 (PE):  per q, gray chunk stationary, {1,cos,sin}-rowsum weights
                -> PSUM [128=(p,cc), 192=(q,type,tI_l)]
  stage2 (PE):  4 block-diag DFT matmuls -> [64=(p,k), 192] cos/sin x g0/g1
  magnitude (ACT/DVE): sqrt of sum of squares, x0.5 -> V [64=(p,k), 64=tI]
  stage3 (PE):  V (stationary) @ p-masked replicated W8 -> outsm[:, 512p]
  store (1 DMA): outsm sample half [64=(tI), (p,j)=4096] -> ys rows 8*tI+p
"""

import sys

sys.path.insert(0, "/opt/trn_rl_repo")

import numpy as np

from concourse import bacc
import concourse.mybir as mybir
from concourse.tile import TileContext
from concourse.bass_utils import run_bass_kernel_spmd

N_CORES = 8
B_FULL = 64
B_CORE = B_FULL // N_CORES  # 8 samples per core
H = W = 512
K = 8  # fft tile size
NQ = 4  # 128-row chunks per image
NC_COLS = 128  # needed image columns per row (16 of every 64)
DT = mybir.dt.float32

# image columns ever sampled by the width resize: 64p+24 .. 64p+39
COLS = (np.arange(K)[:, None] * 64 + 24 + np.arange(16)[None, :]).reshape(-1)


# ----------------------------------------------------------------------------
# host-side constants
# ----------------------------------------------------------------------------
def _make_constants():
    j = np.arange(K)
    cosr = np.cos(2 * np.pi * j / K)
    sinr = np.sin(2 * np.pi * j / K)

    # wred [128, 48]: free = 16*type + tI_l; type 0: plain sum, 1: cos, 2: sin
    wtypes = [np.ones(K), cosr, sinr]
    wred = np.zeros((128, 48), dtype=np.float32)
    for ty in range(3):
        for t in range(16):
            rows = slice(8 * t, 8 * t + 8)
            wred[rows, 16 * ty + t] = wtypes[ty]

    # dft matrices C[v,c] = cos(2pi v c/8), S[v,c] = sin(2pi v c/8)
    v = np.arange(K)
    C8 = np.cos(2 * np.pi * np.outer(v, j) / K).astype(np.float32)
    S8 = np.sin(2 * np.pi * np.outer(v, j) / K).astype(np.float32)

    # dftc [128, 320]: 5 blocks of 64 cols: [C*g0 | S*g0 | -S*g0 | C*g1 | S*g1]
    # partition = 16p + cc (cc in 0..15, g = cc//8); out col = 64*s + 8p + k
    dftc = np.zeros((128, 320), dtype=np.float32)
    for p in range(8):
        for cc in range(16):
            g, c = divmod(cc, 8)
            for k in range(8):
                cv, sv = C8[k, c], S8[k, c]
                if g == 0:
                    dftc[16 * p + cc, 0 + 8 * p + k] = cv
                    dftc[16 * p + cc, 64 + 8 * p + k] = sv
                    dftc[16 * p + cc, 128 + 8 * p + k] = -sv
                else:
                    dftc[16 * p + cc, 192 + 8 * p + k] = cv
                    dftc[16 * p + cc, 256 + 8 * p + k] = sv

    # W8 [8, 512]: bilinear width resize 8 -> 512 (align_corners=False)
    src = (np.arange(W) + 0.5) * (K / W) - 0.5
    src = np.clip(src, 0.0, K - 1.0)
    i0 = np.floor(src).astype(np.int64)
    i1 = np.minimum(i0 + 1, K - 1)
    fr = (src - i0).astype(np.float32)
    W8 = np.zeros((K, W), dtype=np.float32)
    for jj in range(W):
        W8[i0[jj], jj] += 1.0 - fr[jj]
        W8[i1[jj], jj] += fr[jj]

    # wrep [64, 8*512]: block p holds W8 on partitions 8p..8p+7, zero elsewhere
    wrep = np.zeros((64, 8 * W), dtype=np.float32)
    for p in range(8):
        wrep[8 * p : 8 * p + 8, W * p : W * p + W] = W8

    return wred, dftc, wrep


_WRED, _DFTC, _WREP = _make_constants()


def _prep_xs(x: np.ndarray) -> np.ndarray:
    """[B, 3, H, W] f32 -> [B, H, 3, 128] f32: only needed cols, row-major."""
    return np.ascontiguousarray(x[:, :, :, COLS].transpose(0, 2, 1, 3))


def _core_in_maps(x: np.ndarray) -> list[dict]:
    xg = _prep_xs(np.ascontiguousarray(x, dtype=np.float32))
    return [
        {
            "xs": xg[c * B_CORE : (c + 1) * B_CORE],
            "wred": _WRED,
            "dftc": _DFTC,
            "wrep": _WREP,
        }
        for c in range(N_CORES)
    ]


# ----------------------------------------------------------------------------
# bass program (identical on all cores; per-core inputs differ)
# ----------------------------------------------------------------------------
def _build_program(repeat=1):
    nc = bacc.Bacc()

    xs = nc.declare_dram_parameter("xs", [B_CORE, H, 3, NC_COLS], DT, isOutput=False)
    wred_d = nc.declare_dram_parameter("wred", [128, 48], DT, isOutput=False)
    dftc_d = nc.declare_dram_parameter("dftc", [128, 320], DT, isOutput=False)
    wrep_d = nc.declare_dram_parameter("wrep", [64, 8 * W], mybir.dt.float32r, isOutput=False)
    ys = nc.declare_dram_parameter("ys", [B_CORE, 1, H, W], DT, isOutput=True)

    with TileContext(nc) as tc:
        with (
            tc.tile_pool(name="consts", bufs=1) as cpool,
            tc.tile_pool(name="xin", bufs=3) as xpool,
            tc.tile_pool(name="mid", bufs=2) as mpool,
            tc.tile_pool(name="outp", bufs=2) as opool,
            tc.tile_pool(name="ps1", bufs=3, space="PSUM") as ps1pool,
            tc.tile_pool(name="ps2", bufs=2, space="PSUM") as ps2pool,
            tc.tile_pool(name="ps3", bufs=3, space="PSUM") as ps3pool,
        ):
            wred_sb = cpool.tile([128, 48], DT, tag="wred")
            nc.sync.dma_start(wred_sb[:], wred_d[:])
            dftc_sb = cpool.tile([128, 320], DT, tag="dftc")
            nc.sync.dma_start(dftc_sb[:], dftc_d[:])
            wrep_sb = cpool.tile([64, 8 * W], mybir.dt.float32r, tag="wrep")
            nc.scalar.dma_start(wrep_sb[:], wrep_d[:])

            rep_ctx = tc.For_i(0, repeat, 1) if repeat > 1 else None
            if rep_ctx is not None:
                rep_ctx.__enter__()
            for bg2 in range(B_CORE // 2):
                # ---- two samples per iteration: stage2/3 run at 128-wide ----
                xn = []
                for smp in range(2):
                    bg = 2 * bg2 + smp
                    xneed = xpool.tile([128, NQ * 3 * NC_COLS], DT, tag=f"xn{smp}")
                    xneedv = xneed.rearrange(
                        "p (q ch c) -> p q ch c", q=NQ, ch=3
                    )
                    xsrc = xs[bg].rearrange("(q p) ch c -> p q ch c", p=128)
                    (nc.sync if smp == 0 else nc.scalar).dma_start(xneedv[:], xsrc)
                    xn.append(xneedv)

                # ---- gray conversion + stage 1 row reductions per sample ----
                rhs2 = mpool.tile([128, 2 * 192], DT, tag="rhs2")
                xgray = mpool.tile([128, 2 * 512], DT, tag="xgray")
                for smp in range(2):
                    xg = xgray[:, 512 * smp : 512 * smp + 512]
                    rch = xn[smp][:, :, 0]
                    gch = xn[smp][:, :, 1]
                    bch = xn[smp][:, :, 2]
                    t1 = mpool.tile([128, 512], DT, tag=f"t1{smp}")
                    t1v = t1.rearrange("p (q c) -> p q c", q=NQ)
                    nc.scalar.activation(
                        t1v[:], gch,
                        mybir.ActivationFunctionType.Copy, scale=0.587,
                    )
                    t2 = mpool.tile([128, 512], DT, tag=f"t2{smp}")
                    t2v = t2.rearrange("p (q c) -> p q c", q=NQ)
                    nc.vector.scalar_tensor_tensor(
                        t2v[:], rch, 0.299, t1v[:],
                        mybir.AluOpType.mult, mybir.AluOpType.add,
                    )
                    xgv4 = xg.rearrange("p (q c) -> p q c", q=NQ)
                    nc.vector.scalar_tensor_tensor(
                        xgv4[:], bch, 0.114, t2v[:],
                        mybir.AluOpType.mult, mybir.AluOpType.add,
                    )
                xgv = xgray.rearrange("p (sq c) -> p sq c", c=128)
                for smp in range(2):
                    ps1 = ps1pool.tile([128, 192], DT, tag="ps1")
                    for q in range(NQ):
                        nc.tensor.matmul(
                            ps1[:, 48 * q : 48 * q + 48],
                            xgv[:, 4 * smp + q],
                            wred_sb[:],
                            start=True, stop=True,
                        )
                    nc.vector.tensor_copy(
                        rhs2[:, 192 * smp : 192 * smp + 192], ps1[:]
                    )

                # ---- stage 2: DFT + height-blend fused via PSUM accumulation
                # psQ [64=(p,k), 512] = [R3 | I3 | R4 | I4] blocks of (smp, q, tI_l)
                rhs2v = rhs2.rearrange("p (s q blk) -> p s q blk", s=2, q=NQ)
                selA = rhs2v[:, :, :, 0:16]
                selCr = rhs2v[:, :, :, 16:32]
                selCi = rhs2v[:, :, :, 32:48]
                psQ = ps2pool.tile([64, 512], DT, tag="psQ")
                C0 = dftc_sb[:, 0:64]
                S0 = dftc_sb[:, 64:128]
                S0n = dftc_sb[:, 128:192]
                C1 = dftc_sb[:, 192:256]
                S1 = dftc_sb[:, 256:320]
                nc.tensor.matmul(psQ[:, 0:128], C0, selCr, start=True, stop=False)
                nc.tensor.matmul(psQ[:, 0:128], S0, selCi, start=False, stop=True)
                nc.tensor.matmul(psQ[:, 128:256], C0, selCi, start=True, stop=False)
                nc.tensor.matmul(psQ[:, 128:256], S0n, selCr, start=False, stop=True)
                nc.tensor.matmul(psQ[:, 256:384], C1, selA, start=True, stop=True)
                nc.tensor.matmul(psQ[:, 384:512], S1, selA, start=True, stop=True)

                # magnitudes: m = 0.5*sqrt(re^2 + im^2), [64, 128] each
                Sq = mybir.ActivationFunctionType.Square
                p3 = mpool.tile([64, 128], DT, tag="p3")
                nc.scalar.activation(p3[:], psQ[:, 0:128], Sq)
                q3 = mpool.tile([64, 128], DT, tag="q3")
                nc.scalar.activation(q3[:], psQ[:, 128:256], Sq)
                s3 = mpool.tile([64, 128], DT, tag="s3")
                nc.vector.tensor_add(s3[:], p3[:], q3[:])
                m3 = mpool.tile([64, 128], DT, tag="m3")
                nc.scalar.activation(
                    m3[:], s3[:], mybir.ActivationFunctionType.Sqrt, scale=0.25
                )
                p4 = mpool.tile([64, 128], DT, tag="p4")
                nc.scalar.activation(p4[:], psQ[:, 256:384], Sq)
                q4 = mpool.tile([64, 128], DT, tag="q4")
                nc.scalar.activation(q4[:], psQ[:, 384:512], Sq)
                s4 = mpool.tile([64, 128], DT, tag="s4")
                nc.vector.tensor_add(s4[:], p4[:], q4[:])
                m4 = mpool.tile([64, 128], DT, tag="m4")
                nc.scalar.activation(
                    m4[:], s4[:], mybir.ActivationFunctionType.Sqrt, scale=0.25
                )
                vt = mpool.tile([64, 128], mybir.dt.float32r, tag="vt")
                nc.vector.tensor_add(vt[:], m3[:], m4[:])

                # ---- stage 3: width resize; out partitions = (smp, tI) ----
                # outsm free layout: col 512*p + j holds out row (8*tI + p), col j
                outsm = opool.tile([128, K * W], DT, tag="outsm")
                for p in range(8):
                    ps3 = ps3pool.tile([128, W], DT, tag="ps3")
                    nc.tensor.matmul(
                        ps3[:],
                        vt[:],
                        wrep_sb[:, W * p : W * p + W],
                        start=True, stop=True,
                    )
                    dst = outsm[:, W * p : W * p + W]
                    if p % 2 == 0:
                        nc.vector.tensor_copy(dst, ps3[:])
                    else:
                        nc.scalar.copy(dst, ps3[:])

                # one store per sample: rows 8*tI + p are 16KB-contiguous per tI
                for smp in range(2):
                    bg = 2 * bg2 + smp
                    ydst = ys[bg, 0].rearrange("(t r) j -> t (r j)", r=8)
                    (nc.sync if smp == 0 else nc.scalar).dma_start(
                        ydst, outsm[64 * smp : 64 * smp + 64, :]
                    )

            if rep_ctx is not None:
                rep_ctx.__exit__(None, None, None)

    nc.compile()
    return nc


_NC = None


def _get_program():
    global _NC
    if _NC is None:
        _NC = _build_program()
    return _NC


def kernel(x: np.ndarray) -> np.ndarray:
    assert x.shape == (B_FULL, 3, H, W), x.shape
    nc = _get_program()
    in_maps = _core_in_maps(x)
    res = run_bass_kernel_spmd(nc, in_maps, core_ids=list(range(N_CORES)))
    out = np.concatenate([res.results[c]["ys"] for c in range(N_CORES)], axis=0)
    return out


# revision 23
# speedup vs baseline: 1.3291x; 1.3291x over previous
"""Trainium2 Bass kernel for nn_DCTHighPass.

Reference computation (per sample, 512x512 RGB image):
  gray = 0.299 R + 0.587 G + 0.114 B
  tiles = 8x8 blocks of gray (64x64 tiles, row-major (ti, tj))
  mag = |fft2(tile)|
  (buggy mask touches only batch 3:6 / fft rows 3:6 -> never sampled below)
  img = mag tiles stacked into [4096*8, 8]
  out = bilinear_resize(img, 512, 512)

Key reduction: the height downsample (32768 -> 512, factor 64) samples only
input rows 64i+31 and 64i+32 with weight 0.5 each, i.e. fft-row 7 of tile
(ti=i//8, tj=8*(i%8)+3) and fft-row 0 of tile (ti, tj=8*(i%8)+4).  fft rows
0/7 of an 8x8 real tile need only three 8-weight row reductions of the tile
(plain sum, cos, sin), followed by an 8-point DFT along columns.  The width
upsample (8 -> 512) is a fixed [8,512] matrix.  So per output row i:
  v = 0.5*(|DFT(cos/sin rowsums of tile tj3)| + |DFT(colsum of tile tj4)|)
  out[i, :] = v @ W8
Only image columns 64p+24 .. 64p+39 (p = i%8) are ever used, so the host
pre-gathers exactly those 128 columns per row into a [B, H, 3, 128]
row-major tensor; every DMA run is then >= 1536B contiguous (full DMA-bus
rate; the original 64B gather runs paid the sub-512B 2x latency penalty).

Pipeline per sample (per core, batch of 8 samples):
  load  (1 DMA):  xs[smp] -> SBUF [128=(row%128), (q, ch, c)=1536]
  gray  (ACT/DVE): 0.299 R + 0.587 G + 0.114 B -> [128, (q c)=512]
  stage1 (PE):  per q, gray chunk stationary, {1,cos,sin}-rowsum weights
                -> PSUM [128=(p,cc), 192=(q,type,tI_l)]
  stage2 (PE):  4 block-diag DFT matmuls -> [64=(p,k), 192] cos/sin x g0/g1
  magnitude (ACT/DVE): sqrt of sum of squares, x0.5 -> V [64=(p,k), 64=tI]
  stage3 (PE):  V (stationary) @ p-masked replicated W8 -> outsm[:, 512p]
  store (1 DMA): outsm sample half [64=(tI), (p,j)=4096] -> ys rows 8*tI+p
"""

import sys

sys.path.insert(0, "/opt/trn_rl_repo")

import numpy as np

from concourse import bacc
import concourse.mybir as mybir
from concourse.tile import TileContext
from concourse.bass_utils import run_bass_kernel_spmd

N_CORES = 8
B_FULL = 64
B_CORE = B_FULL // N_CORES  # 8 samples per core
H = W = 512
K = 8  # fft tile size
NQ = 4  # 128-row chunks per image
NC_COLS = 128  # needed image columns per row (16 of every 64)
DT = mybir.dt.float32

# image columns ever sampled by the width resize: 64p+24 .. 64p+39
COLS = (np.arange(K)[:, None] * 64 + 24 + np.arange(16)[None, :]).reshape(-1)


# ----------------------------------------------------------------------------
# host-side constants
# ----------------------------------------------------------------------------
def _make_constants():
    j = np.arange(K)
    cosr = np.cos(2 * np.pi * j / K)
    sinr = np.sin(2 * np.pi * j / K)

    # wred48 [128, 48]: free = 16*type + tI_l; type 0: plain sum, 1: cos, 2: sin
    # wred144 [128, 144]: same but per-channel gray-coefficient scaled blocks
    wtypes = [np.ones(K), cosr, sinr]
    wred = np.zeros((128, 48), dtype=np.float32)
    for ty in range(3):
        for t in range(16):
            rows = slice(8 * t, 8 * t + 8)
            wred[rows, 16 * ty + t] = wtypes[ty]
    coef = [0.299, 0.587, 0.114]
    wred144 = np.zeros((128, 144), dtype=np.float32)
    for ch in range(3):
        wred144[:, 48 * ch : 48 * ch + 48] = coef[ch] * wred

    # dft matrices C[v,c] = cos(2pi v c/8), S[v,c] = sin(2pi v c/8)
    v = np.arange(K)
    C8 = np.cos(2 * np.pi * np.outer(v, j) / K).astype(np.float32)
    S8 = np.sin(2 * np.pi * np.outer(v, j) / K).astype(np.float32)

    # dftc [128, 320]: 5 blocks of 64 cols: [C*g0 | S*g0 | -S*g0 | C*g1 | S*g1]
    # partition = 16p + cc (cc in 0..15, g = cc//8); out col = 64*s + 8p + k
    dftc = np.zeros((128, 320), dtype=np.float32)
    for p in range(8):
        for cc in range(16):
            g, c = divmod(cc, 8)
            for k in range(8):
                cv, sv = C8[k, c], S8[k, c]
                if g == 0:
                    dftc[16 * p + cc, 0 + 8 * p + k] = cv
                    dftc[16 * p + cc, 64 + 8 * p + k] = sv
                    dftc[16 * p + cc, 128 + 8 * p + k] = -sv
                else:
                    dftc[16 * p + cc, 192 + 8 * p + k] = cv
                    dftc[16 * p + cc, 256 + 8 * p + k] = sv

    # W8 [8, 512]: bilinear width resize 8 -> 512 (align_corners=False)
    src = (np.arange(W) + 0.5) * (K / W) - 0.5
    src = np.clip(src, 0.0, K - 1.0)
    i0 = np.floor(src).astype(np.int64)
    i1 = np.minimum(i0 + 1, K - 1)
    fr = (src - i0).astype(np.float32)
    W8 = np.zeros((K, W), dtype=np.float32)
    for jj in range(W):
        W8[i0[jj], jj] += 1.0 - fr[jj]
        W8[i1[jj], jj] += fr[jj]

    # wrep [64, 8*512]: block p holds W8 on partitions 8p..8p+7, zero elsewhere
    wrep = np.zeros((64, 8 * W), dtype=np.float32)
    for p in range(8):
        wrep[8 * p : 8 * p + 8, W * p : W * p + W] = W8

    return dict(wred48=wred, wred144=wred144, dftc=dftc, wrep=wrep)


_CONSTS = _make_constants()
_DFTC = _CONSTS["dftc"]
_WREP = _CONSTS["wrep"]


def _in_np_dtype(cfg):
    import ml_dtypes

    return ml_dtypes.bfloat16 if cfg["in_bf16"] else np.float32


def _core_in_maps(x: np.ndarray, cfg=None) -> list[dict]:
    cfg = {**CFG, **(cfg or {})}
    xg = np.ascontiguousarray(
        x[:, :, :, COLS].transpose(0, 2, 1, 3), dtype=_in_np_dtype(cfg)
    )
    wred = _CONSTS["wred144"] if cfg["gray_fold"] else _CONSTS["wred48"]
    if cfg["gray_fold"] and cfg["in_bf16"]:
        wred = wred.astype(_in_np_dtype(cfg))
    return [
        {
            "xs": xg[c * B_CORE : (c + 1) * B_CORE],
            "wred": wred,
            "dftc": _DFTC,
            "wrep": _WREP,
        }
        for c in range(N_CORES)
    ]


# ----------------------------------------------------------------------------
# bass program (identical on all cores; per-core inputs differ)
# ----------------------------------------------------------------------------
CFG = dict(
    load_parts=1,   # DMAs per sample for the input load: 1, 3 (per ch), 12 (ch,q)
    store_parts=1,  # DMAs per sample for the output store: 1, 2, 4 (t ranges)
    load_qs="sc",   # queue cycle for loads: s=sync, c=scalar, v=vector, g=gpsimd
    store_qs="sc",  # queue cycle for stores
    in_bf16=True,   # ship x columns as bf16 (halves load traffic)
    out_bf16=True,  # store ys as bf16, upcast on host (halves store traffic)
    gray_fold=True, # fold gray coefficients into stage-1 matmul weights
    skip_loads=False,   # timing diagnostics only -- breaks correctness
    skip_stores=False,
    skip_compute=False,
)


def _emit_stores(nc, cfg, sqs, ys, bg2, outsm, si):
    # stores: rows 8*tI + p are 16KB-contiguous per tI; split over t ranges
    # if store_parts > 1
    if cfg["skip_stores"]:
        return
    sp = cfg["store_parts"]
    tchunk = 64 // sp
    for smp in range(2):
        bg = 2 * bg2 + smp
        ydst = ys[bg, 0].rearrange("(t r) j -> t (r j)", r=8)
        for u in range(sp):
            t0 = u * tchunk
            sqs[si % len(sqs)].dma_start(
                ydst[t0 : t0 + tchunk],
                outsm[64 * smp + t0 : 64 * smp + t0 + tchunk, :],
            )
            si += 1


def _build_program(repeat=1, cfg=None):
    cfg = {**CFG, **(cfg or {})}
    nc = bacc.Bacc()

    dt_in = mybir.dt.bfloat16 if cfg["in_bf16"] else DT
    dt_out = mybir.dt.bfloat16 if cfg["out_bf16"] else DT
    wred_shape = [128, 144] if cfg["gray_fold"] else [128, 48]
    dt_wred = dt_in if cfg["gray_fold"] else DT

    xs = nc.declare_dram_parameter("xs", [B_CORE, H, 3, NC_COLS], dt_in, isOutput=False)
    wred_d = nc.declare_dram_parameter("wred", wred_shape, dt_wred, isOutput=False)
    dftc_d = nc.declare_dram_parameter("dftc", [128, 320], DT, isOutput=False)
    wrep_d = nc.declare_dram_parameter("wrep", [64, 8 * W], mybir.dt.float32r, isOutput=False)
    ys = nc.declare_dram_parameter("ys", [B_CORE, 1, H, W], dt_out, isOutput=True)

    with TileContext(nc) as tc:
        with (
            tc.tile_pool(name="consts", bufs=1) as cpool,
            tc.tile_pool(name="xin", bufs=3) as xpool,
            tc.tile_pool(name="mid", bufs=2) as mpool,
            tc.tile_pool(name="outp", bufs=2) as opool,
            tc.tile_pool(name="ps1", bufs=3, space="PSUM") as ps1pool,
            tc.tile_pool(name="ps2", bufs=2, space="PSUM") as ps2pool,
            tc.tile_pool(name="ps3", bufs=3, space="PSUM") as ps3pool,
        ):
            wred_sb = cpool.tile(wred_shape, dt_wred, tag="wred")
            nc.sync.dma_start(wred_sb[:], wred_d[:])
            dftc_sb = cpool.tile([128, 320], DT, tag="dftc")
            nc.sync.dma_start(dftc_sb[:], dftc_d[:])
            wrep_sb = cpool.tile([64, 8 * W], mybir.dt.float32r, tag="wrep")
            nc.scalar.dma_start(wrep_sb[:], wrep_d[:])

            queues = dict(s=nc.sync, c=nc.scalar, v=nc.vector, g=nc.gpsimd)
            lqs = [queues[q] for q in cfg["load_qs"]]
            sqs = [queues[q] for q in cfg["store_qs"]]
            li = si = 0

            rep_ctx = tc.For_i(0, repeat, 1) if repeat > 1 else None
            if rep_ctx is not None:
                rep_ctx.__enter__()
            for bg2 in range(B_CORE // 2):
                # ---- two samples per iteration: stage2/3 run at 128-wide ----
                xn = []
                for smp in range(2):
                    bg = 2 * bg2 + smp
                    xneed = xpool.tile([128, NQ * 3 * NC_COLS], dt_in, tag=f"xn{smp}")
                    xneedv = xneed.rearrange(
                        "p (q ch c) -> p q ch c", q=NQ, ch=3
                    )
                    xsrc = xs[bg].rearrange("(q p) ch c -> p q ch c", p=128)
                    if cfg["skip_loads"]:
                        nc.vector.memset(xneed[:], 0.5)
                        xn.append(xneedv)
                        continue
                    if cfg["load_parts"] == 1:
                        lqs[li % len(lqs)].dma_start(xneedv[:], xsrc)
                        li += 1
                    elif cfg["load_parts"] == 3:
                        for ch in range(3):
                            lqs[li % len(lqs)].dma_start(
                                xneedv[:, :, ch], xsrc[:, :, ch]
                            )
                            li += 1
                    else:
                        for ch in range(3):
                            for q in range(NQ):
                                lqs[li % len(lqs)].dma_start(
                                    xneedv[:, q, ch], xsrc[:, q, ch]
                                )
                                li += 1
                    xn.append(xneedv)

                # ---- gray conversion + stage 1 row reductions per sample ----
                outsm = opool.tile([128, K * W], dt_out, tag="outsm")
                if cfg["skip_compute"]:
                    nc.vector.memset(outsm[:, 0:8], 0.5)
                    _emit_stores(nc, cfg, sqs, ys, bg2, outsm, si)
                    si += 2 * cfg["store_parts"]
                    continue
                rhs2 = mpool.tile([128, 2 * 192], DT, tag="rhs2")
                if cfg["gray_fold"]:
                    # stage-1 matmuls read raw channels; gray coefficients are
                    # folded into per-channel weight blocks (PSUM-accumulated)
                    for smp in range(2):
                        xflat = xn[smp].rearrange("p q ch c -> p (q ch) c")
                        ps1 = ps1pool.tile([128, 192], DT, tag="ps1")
                        for q in range(NQ):
                            for ch in range(3):
                                nc.tensor.matmul(
                                    ps1[:, 48 * q : 48 * q + 48],
                                    xflat[:, 3 * q + ch],
                                    wred_sb[:, 48 * ch : 48 * ch + 48],
                                    start=(ch == 0),
                                    stop=(ch == 2),
                                )
                        nc.vector.tensor_copy(
                            rhs2[:, 192 * smp : 192 * smp + 192], ps1[:]
                        )
                else:
                    xgray = mpool.tile([128, 2 * 512], DT, tag="xgray")
                    for smp in range(2):
                        xg = xgray[:, 512 * smp : 512 * smp + 512]
                        rch = xn[smp][:, :, 0]
                        gch = xn[smp][:, :, 1]
                        bch = xn[smp][:, :, 2]
                        t1 = mpool.tile([128, 512], DT, tag=f"t1{smp}")
                        t1v = t1.rearrange("p (q c) -> p q c", q=NQ)
                        nc.scalar.activation(
                            t1v[:], gch,
                            mybir.ActivationFunctionType.Copy, scale=0.587,
                        )
                        t2 = mpool.tile([128, 512], DT, tag=f"t2{smp}")
                        t2v = t2.rearrange("p (q c) -> p q c", q=NQ)
                        nc.vector.scalar_tensor_tensor(
                            t2v[:], rch, 0.299, t1v[:],
                            mybir.AluOpType.mult, mybir.AluOpType.add,
                        )
                        xgv4 = xg.rearrange("p (q c) -> p q c", q=NQ)
                        nc.vector.scalar_tensor_tensor(
                            xgv4[:], bch, 0.114, t2v[:],
                            mybir.AluOpType.mult, mybir.AluOpType.add,
                        )
                    xgv = xgray.rearrange("p (sq c) -> p sq c", c=128)
                    for smp in range(2):
                        ps1 = ps1pool.tile([128, 192], DT, tag="ps1")
                        for q in range(NQ):
                            nc.tensor.matmul(
                                ps1[:, 48 * q : 48 * q + 48],
                                xgv[:, 4 * smp + q],
                                wred_sb[:],
                                start=True, stop=True,
                            )
                        nc.vector.tensor_copy(
                            rhs2[:, 192 * smp : 192 * smp + 192], ps1[:]
                        )

                # ---- stage 2: DFT + height-blend fused via PSUM accumulation
                # psQ [64=(p,k), 512] = [R3 | I3 | R4 | I4] blocks of (smp, q, tI_l)
                rhs2v = rhs2.rearrange("p (s q blk) -> p s q blk", s=2, q=NQ)
                selA = rhs2v[:, :, :, 0:16]
                selCr = rhs2v[:, :, :, 16:32]
                selCi = rhs2v[:, :, :, 32:48]
                psQ = ps2pool.tile([64, 512], DT, tag="psQ")
                C0 = dftc_sb[:, 0:64]
                S0 = dftc_sb[:, 64:128]
                S0n = dftc_sb[:, 128:192]
                C1 = dftc_sb[:, 192:256]
                S1 = dftc_sb[:, 256:320]
                nc.tensor.matmul(psQ[:, 0:128], C0, selCr, start=True, stop=False)
                nc.tensor.matmul(psQ[:, 0:128], S0, selCi, start=False, stop=True)
                nc.tensor.matmul(psQ[:, 128:256], C0, selCi, start=True, stop=False)
                nc.tensor.matmul(psQ[:, 128:256], S0n, selCr, start=False, stop=True)
                nc.tensor.matmul(psQ[:, 256:384], C1, selA, start=True, stop=True)
                nc.tensor.matmul(psQ[:, 384:512], S1, selA, start=True, stop=True)

                # magnitudes: m = 0.5*sqrt(re^2 + im^2), [64, 128] each
                Sq = mybir.ActivationFunctionType.Square
                p3 = mpool.tile([64, 128], DT, tag="p3")
                nc.scalar.activation(p3[:], psQ[:, 0:128], Sq)
                q3 = mpool.tile([64, 128], DT, tag="q3")
                nc.scalar.activation(q3[:], psQ[:, 128:256], Sq)
                s3 = mpool.tile([64, 128], DT, tag="s3")
                nc.vector.tensor_add(s3[:], p3[:], q3[:])
                m3 = mpool.tile([64, 128], DT, tag="m3")
                nc.scalar.activation(
                    m3[:], s3[:], mybir.ActivationFunctionType.Sqrt, scale=0.25
                )
                p4 = mpool.tile([64, 128], DT, tag="p4")
                nc.scalar.activation(p4[:], psQ[:, 256:384], Sq)
                q4 = mpool.tile([64, 128], DT, tag="q4")
                nc.scalar.activation(q4[:], psQ[:, 384:512], Sq)
                s4 = mpool.tile([64, 128], DT, tag="s4")
                nc.vector.tensor_add(s4[:], p4[:], q4[:])
                m4 = mpool.tile([64, 128], DT, tag="m4")
                nc.scalar.activation(
                    m4[:], s4[:], mybir.ActivationFunctionType.Sqrt, scale=0.25
                )
                vt = mpool.tile([64, 128], mybir.dt.float32r, tag="vt")
                nc.vector.tensor_add(vt[:], m3[:], m4[:])

                # ---- stage 3: width resize; out partitions = (smp, tI) ----
                # outsm free layout: col 512*p + j holds out row (8*tI + p), col j
                for p in range(8):
                    ps3 = ps3pool.tile([128, W], DT, tag="ps3")
                    nc.tensor.matmul(
                        ps3[:],
                        vt[:],
                        wrep_sb[:, W * p : W * p + W],
                        start=True, stop=True,
                    )
                    dst = outsm[:, W * p : W * p + W]
                    if p % 2 == 0:
                        nc.vector.tensor_copy(dst, ps3[:])
                    else:
                        nc.scalar.copy(dst, ps3[:])

                _emit_stores(nc, cfg, sqs, ys, bg2, outsm, si)
                si += 2 * cfg["store_parts"]

            if rep_ctx is not None:
                rep_ctx.__exit__(None, None, None)

    nc.compile()
    return nc


_NC = None


def _get_program():
    global _NC
    if _NC is None:
        _NC = _build_program()
    return _NC


def kernel(x: np.ndarray) -> np.ndarray:
    assert x.shape == (B_FULL, 3, H, W), x.shape
    nc = _get_program()
    in_maps = _core_in_maps(x)
    res = run_bass_kernel_spmd(nc, in_maps, core_ids=list(range(N_CORES)))
    out = np.concatenate([res.results[c]["ys"] for c in range(N_CORES)], axis=0)
    return np.ascontiguousarray(out, dtype=np.float32)


# revision 44
# speedup vs baseline: 1.7264x; 1.2989x over previous
"""Trainium2 Bass kernel for nn_DCTHighPass.

Reference computation (per sample, 512x512 RGB image):
  gray = 0.299 R + 0.587 G + 0.114 B
  tiles = 8x8 blocks of gray (64x64 tiles, row-major (ti, tj))
  mag = |fft2(tile)|
  (buggy mask touches only batch 3:6 / fft rows 3:6 -> never sampled below)
  img = mag tiles stacked into [4096*8, 8]
  out = bilinear_resize(img, 512, 512)

Key reduction: the height downsample (32768 -> 512, factor 64) samples only
input rows 64i+31 and 64i+32 with weight 0.5 each, i.e. fft-row 7 of tile
(ti=i//8, tj=8*(i%8)+3) and fft-row 0 of tile (ti, tj=8*(i%8)+4).  fft rows
0/7 of an 8x8 real tile need only three 8-weight row reductions of the tile
(plain sum, cos, sin), followed by an 8-point DFT along columns.  The width
upsample (8 -> 512) is a fixed [8,512] matrix.  So per output row i:
  v = 0.5*(|DFT(cos/sin rowsums of tile tj3)| + |DFT(colsum of tile tj4)|)
  out[i, :] = v @ W8
Only image columns 64p+24 .. 64p+39 (p = i%8) are ever used, so the host
pre-gathers exactly those 128 columns per row into a [B, H, 3, 128]
row-major tensor; every DMA run is then >= 1536B contiguous (full DMA-bus
rate; the original 64B gather runs paid the sub-512B 2x latency penalty).

Pipeline per sample (per core, batch of 8 samples):
  load  (1 DMA):  xs[smp] -> SBUF [128=(row%128), (q, ch, c)=1536]
  gray  (ACT/DVE): 0.299 R + 0.587 G + 0.114 B -> [128, (q c)=512]
  stage1 (PE):  per q, gray chunk stationary, {1,cos,sin}-rowsum weights
                -> PSUM [128=(p,cc), 192=(q,type,tI_l)]
  stage2 (PE):  4 block-diag DFT matmuls -> [64=(p,k), 192] cos/sin x g0/g1
  magnitude (ACT/DVE): sqrt of sum of squares, x0.5 -> V [64=(p,k), 64=tI]
  stage3 (PE):  V (stationary) @ p-masked replicated W8 -> outsm[:, 512p]
  store (1 DMA): outsm sample half [64=(tI), (p,j)=4096] -> ys rows 8*tI+p
"""

import sys

sys.path.insert(0, "/opt/trn_rl_repo")

import numpy as np

from concourse import bacc
import concourse.mybir as mybir
from concourse.tile import TileContext
from concourse.bass_utils import run_bass_kernel_spmd

N_CORES = 8
B_FULL = 64
B_CORE = B_FULL // N_CORES  # 8 samples per core
H = W = 512
K = 8  # fft tile size
NQ = 4  # 128-row chunks per image
NC_COLS = 128  # needed image columns per row (16 of every 64)
DT = mybir.dt.float32

# image columns ever sampled by the width resize: 64p+24 .. 64p+39
COLS = (np.arange(K)[:, None] * 64 + 24 + np.arange(16)[None, :]).reshape(-1)


# ----------------------------------------------------------------------------
# host-side constants
# ----------------------------------------------------------------------------
def _make_constants():
    j = np.arange(K)
    cosr = np.cos(2 * np.pi * j / K)
    sinr = np.sin(2 * np.pi * j / K)

    # wred48 [128, 48]: free = 16*type + tI_l; type 0: plain sum, 1: cos, 2: sin
    # wred144 [128, 144]: same but per-channel gray-coefficient scaled blocks
    wtypes = [np.ones(K), cosr, sinr]
    wred = np.zeros((128, 48), dtype=np.float32)
    for ty in range(3):
        for t in range(16):
            rows = slice(8 * t, 8 * t + 8)
            wred[rows, 16 * ty + t] = wtypes[ty]
    coef = [0.299, 0.587, 0.114]
    wred144 = np.zeros((128, 144), dtype=np.float32)
    for ch in range(3):
        wred144[:, 48 * ch : 48 * ch + 48] = coef[ch] * wred

    # dft matrices C[v,c] = cos(2pi v c/8), S[v,c] = sin(2pi v c/8)
    v = np.arange(K)
    C8 = np.cos(2 * np.pi * np.outer(v, j) / K).astype(np.float32)
    S8 = np.sin(2 * np.pi * np.outer(v, j) / K).astype(np.float32)

    # dftc [128, 320]: 5 blocks of 64 cols: [C*g0 | S*g0 | -S*g0 | C*g1 | S*g1]
    # partition = 16p + cc (cc in 0..15, g = cc//8); out col = 64*s + 8p + k
    dftc = np.zeros((128, 320), dtype=np.float32)
    for p in range(8):
        for cc in range(16):
            g, c = divmod(cc, 8)
            for k in range(8):
                cv, sv = C8[k, c], S8[k, c]
                if g == 0:
                    dftc[16 * p + cc, 0 + 8 * p + k] = cv
                    dftc[16 * p + cc, 64 + 8 * p + k] = sv
                    dftc[16 * p + cc, 128 + 8 * p + k] = -sv
                else:
                    dftc[16 * p + cc, 192 + 8 * p + k] = cv
                    dftc[16 * p + cc, 256 + 8 * p + k] = sv

    # W8 [8, 512]: bilinear width resize 8 -> 512 (align_corners=False)
    src = (np.arange(W) + 0.5) * (K / W) - 0.5
    src = np.clip(src, 0.0, K - 1.0)
    i0 = np.floor(src).astype(np.int64)
    i1 = np.minimum(i0 + 1, K - 1)
    fr = (src - i0).astype(np.float32)
    W8 = np.zeros((K, W), dtype=np.float32)
    for jj in range(W):
        W8[i0[jj], jj] += 1.0 - fr[jj]
        W8[i1[jj], jj] += fr[jj]

    # wrep [64, 8*512]: block p holds W8 on partitions 8p..8p+7, zero elsewhere
    wrep = np.zeros((64, 8 * W), dtype=np.float32)
    for p in range(8):
        wrep[8 * p : 8 * p + 8, W * p : W * p + W] = W8

    return dict(wred48=wred, wred144=wred144, dftc=dftc, wrep=wrep)


_CONSTS = _make_constants()
_DFTC = _CONSTS["dftc"]
_WREP = _CONSTS["wrep"]


def _in_np_dtype(cfg):
    import ml_dtypes

    return ml_dtypes.bfloat16 if cfg["in_bf16"] else np.float32


def _core_in_maps(x: np.ndarray, cfg=None) -> list[dict]:
    cfg = {**CFG, **(cfg or {})}
    xg = np.ascontiguousarray(
        x[:, :, :, COLS].transpose(0, 2, 1, 3), dtype=_in_np_dtype(cfg)
    )
    wred = _CONSTS["wred144"] if cfg["gray_fold"] else _CONSTS["wred48"]
    if cfg["gray_fold"] and cfg["in_bf16"]:
        wred = wred.astype(_in_np_dtype(cfg))
    import ml_dtypes

    wrep = _WREP.astype(ml_dtypes.bfloat16) if cfg["bf3"] else _WREP
    dftc = _DFTC.astype(ml_dtypes.bfloat16) if cfg["bf2"] else _DFTC
    return [
        {
            "xs": xg[c * B_CORE : (c + 1) * B_CORE],
            "wred": wred,
            "dftc": dftc,
            "wrep": wrep,
        }
        for c in range(N_CORES)
    ]


# ----------------------------------------------------------------------------
# bass program (identical on all cores; per-core inputs differ)
# ----------------------------------------------------------------------------
CFG = dict(
    load_pair=False,  # single DMA loads both samples of an iteration
    load_parts=1,   # DMAs per sample for the input load: 1, 3 (per ch), 12 (ch,q)
    store_parts=1,  # DMAs per sample for the output store: 1, 2, 4 (t ranges)
    load_qs="s",    # queue cycle for loads: s=sync, c=scalar, g=gpsimd
    store_qs="s",   # queue cycle for stores (SP queue: no compute to stall)
    in_bf16=True,   # ship x columns as bf16 (halves load traffic)
    out_bf16=True,  # store ys as bf16, upcast on host (halves store traffic)
    gray_fold=True, # fold gray coefficients into stage-1 matmul weights
    mag_compact=False,  # 4-op magnitude chain instead of 9
    wide_ps3=False,     # 2-bank stage-3 PSUM tiles, one copy drains two blocks
    bf3=True,           # stage-3 matmul in bf16 (vt + wrep)
    copy_pat="vava",    # stage-3 drain engines per p (cycled): v=DVE a=Act g=Pool
    xin_bufs=3,
    outp_bufs=2,
    ps_bufs=None,       # (ps1, ps2, ps3) PSUM pool bufs override
    bf2=False,          # stage-2 matmul in bf16 (rhs2 + dftc)
    sq_dve=False,       # magnitude squares on DVE instead of Act
    skip_loads=False,   # timing diagnostics only -- breaks correctness
    skip_stores=False,
    skip_compute=False,
)


def _emit_stores(nc, cfg, sqs, ys, bg2, outsm, si):
    # stores: rows 8*tI + p are 16KB-contiguous per tI; split over t ranges
    # if store_parts > 1
    if cfg["skip_stores"]:
        return
    sp = cfg["store_parts"]
    tchunk = 64 // sp
    for smp in range(2):
        bg = 2 * bg2 + smp
        ydst = ys[bg, 0].rearrange("(t r) j -> t (r j)", r=8)
        for u in range(sp):
            t0 = u * tchunk
            sqs[si % len(sqs)].dma_start(
                ydst[t0 : t0 + tchunk],
                outsm[64 * smp + t0 : 64 * smp + t0 + tchunk, :],
            )
            si += 1


def _build_program(repeat=1, cfg=None):
    cfg = {**CFG, **(cfg or {})}
    nc = bacc.Bacc()

    dt_in = mybir.dt.bfloat16 if cfg["in_bf16"] else DT
    dt_out = mybir.dt.bfloat16 if cfg["out_bf16"] else DT
    wred_shape = [128, 144] if cfg["gray_fold"] else [128, 48]
    dt_wred = dt_in if cfg["gray_fold"] else DT

    dt_w3 = mybir.dt.bfloat16 if cfg["bf3"] else mybir.dt.float32r
    dt_s2 = mybir.dt.bfloat16 if cfg["bf2"] else DT

    xs = nc.declare_dram_parameter("xs", [B_CORE, H, 3, NC_COLS], dt_in, isOutput=False)
    wred_d = nc.declare_dram_parameter("wred", wred_shape, dt_wred, isOutput=False)
    dftc_d = nc.declare_dram_parameter("dftc", [128, 320], dt_s2, isOutput=False)
    wrep_d = nc.declare_dram_parameter("wrep", [64, 8 * W], dt_w3, isOutput=False)
    ys = nc.declare_dram_parameter("ys", [B_CORE, 1, H, W], dt_out, isOutput=True)

    with TileContext(nc) as tc:
        ps1_bufs, ps2_bufs, ps3_bufs = (
            cfg["ps_bufs"]
            if cfg["ps_bufs"]
            else ((2, 2, 2) if cfg["wide_ps3"] else (3, 2, 3))
        )
        with (
            tc.tile_pool(name="consts", bufs=1) as cpool,
            tc.tile_pool(name="xin", bufs=cfg["xin_bufs"]) as xpool,
            tc.tile_pool(name="mid", bufs=2) as mpool,
            tc.tile_pool(name="outp", bufs=cfg["outp_bufs"]) as opool,
            tc.tile_pool(name="ps1", bufs=ps1_bufs, space="PSUM") as ps1pool,
            tc.tile_pool(name="ps2", bufs=ps2_bufs, space="PSUM") as ps2pool,
            tc.tile_pool(name="ps3", bufs=ps3_bufs, space="PSUM") as ps3pool,
        ):
            wred_sb = cpool.tile(wred_shape, dt_wred, tag="wred")
            nc.sync.dma_start(wred_sb[:], wred_d[:])
            dftc_sb = cpool.tile([128, 320], dt_s2, tag="dftc")
            nc.sync.dma_start(dftc_sb[:], dftc_d[:])
            wrep_sb = cpool.tile([64, 8 * W], dt_w3, tag="wrep")
            nc.scalar.dma_start(wrep_sb[:], wrep_d[:])

            queues = dict(s=nc.sync, c=nc.scalar, v=nc.vector, g=nc.gpsimd)
            lqs = [queues[q] for q in cfg["load_qs"]]
            sqs = [queues[q] for q in cfg["store_qs"]]
            li = si = 0

            rep_ctx = tc.For_i(0, repeat, 1) if repeat > 1 else None
            if rep_ctx is not None:
                rep_ctx.__enter__()
            for bg2 in range(B_CORE // 2):
                # ---- two samples per iteration: stage2/3 run at 128-wide ----
                xn = []
                if cfg["load_pair"]:
                    xneed = xpool.tile([128, 2 * NQ * 3 * NC_COLS], dt_in, tag="xnp")
                    xneedv2 = xneed.rearrange(
                        "p (b q ch c) -> p b q ch c", b=2, q=NQ, ch=3
                    )
                    xsrc2 = xs[2 * bg2 : 2 * bg2 + 2].rearrange(
                        "b (q p) ch c -> p b q ch c", p=128
                    )
                    if cfg["skip_loads"]:
                        nc.vector.memset(xneed[:], 0.5)
                    else:
                        lqs[li % len(lqs)].dma_start(xneedv2[:], xsrc2)
                        li += 1
                    xn = [xneedv2[:, 0], xneedv2[:, 1]]
                for smp in range(2) if not cfg["load_pair"] else []:
                    bg = 2 * bg2 + smp
                    xneed = xpool.tile([128, NQ * 3 * NC_COLS], dt_in, tag=f"xn{smp}")
                    xneedv = xneed.rearrange(
                        "p (q ch c) -> p q ch c", q=NQ, ch=3
                    )
                    xsrc = xs[bg].rearrange("(q p) ch c -> p q ch c", p=128)
                    if cfg["skip_loads"]:
                        nc.vector.memset(xneed[:], 0.5)
                        xn.append(xneedv)
                        continue
                    if cfg["load_parts"] == 1:
                        lqs[li % len(lqs)].dma_start(xneedv[:], xsrc)
                        li += 1
                    elif cfg["load_parts"] == 3:
                        for ch in range(3):
                            lqs[li % len(lqs)].dma_start(
                                xneedv[:, :, ch], xsrc[:, :, ch]
                            )
                            li += 1
                    else:
                        for ch in range(3):
                            for q in range(NQ):
                                lqs[li % len(lqs)].dma_start(
                                    xneedv[:, q, ch], xsrc[:, q, ch]
                                )
                                li += 1
                    xn.append(xneedv)

                # ---- gray conversion + stage 1 row reductions per sample ----
                outsm = opool.tile([128, K * W], dt_out, tag="outsm")
                if cfg["skip_compute"]:
                    nc.vector.memset(outsm[:, 0:8], 0.5)
                    _emit_stores(nc, cfg, sqs, ys, bg2, outsm, si)
                    si += 2 * cfg["store_parts"]
                    continue
                rhs2 = mpool.tile([128, 2 * 192], dt_s2, tag="rhs2")
                if cfg["gray_fold"]:
                    # stage-1 matmuls read raw channels; gray coefficients are
                    # folded into per-channel weight blocks (PSUM-accumulated)
                    for smp in range(2):
                        xflat = xn[smp].rearrange("p q ch c -> p (q ch) c")
                        ps1 = ps1pool.tile([128, 192], DT, tag="ps1")
                        for q in range(NQ):
                            for ch in range(3):
                                nc.tensor.matmul(
                                    ps1[:, 48 * q : 48 * q + 48],
                                    xflat[:, 3 * q + ch],
                                    wred_sb[:, 48 * ch : 48 * ch + 48],
                                    start=(ch == 0),
                                    stop=(ch == 2),
                                )
                        nc.vector.tensor_copy(
                            rhs2[:, 192 * smp : 192 * smp + 192], ps1[:]
                        )
                else:
                    xgray = mpool.tile([128, 2 * 512], DT, tag="xgray")
                    for smp in range(2):
                        xg = xgray[:, 512 * smp : 512 * smp + 512]
                        rch = xn[smp][:, :, 0]
                        gch = xn[smp][:, :, 1]
                        bch = xn[smp][:, :, 2]
                        t1 = mpool.tile([128, 512], DT, tag=f"t1{smp}")
                        t1v = t1.rearrange("p (q c) -> p q c", q=NQ)
                        nc.scalar.activation(
                            t1v[:], gch,
                            mybir.ActivationFunctionType.Copy, scale=0.587,
                        )
                        t2 = mpool.tile([128, 512], DT, tag=f"t2{smp}")
                        t2v = t2.rearrange("p (q c) -> p q c", q=NQ)
                        nc.vector.scalar_tensor_tensor(
                            t2v[:], rch, 0.299, t1v[:],
                            mybir.AluOpType.mult, mybir.AluOpType.add,
                        )
                        xgv4 = xg.rearrange("p (q c) -> p q c", q=NQ)
                        nc.vector.scalar_tensor_tensor(
                            xgv4[:], bch, 0.114, t2v[:],
                            mybir.AluOpType.mult, mybir.AluOpType.add,
                        )
                    xgv = xgray.rearrange("p (sq c) -> p sq c", c=128)
                    for smp in range(2):
                        ps1 = ps1pool.tile([128, 192], DT, tag="ps1")
                        for q in range(NQ):
                            nc.tensor.matmul(
                                ps1[:, 48 * q : 48 * q + 48],
                                xgv[:, 4 * smp + q],
                                wred_sb[:],
                                start=True, stop=True,
                            )
                        nc.vector.tensor_copy(
                            rhs2[:, 192 * smp : 192 * smp + 192], ps1[:]
                        )

                # ---- stage 2: DFT + height-blend fused via PSUM accumulation
                # psQ [64=(p,k), 512] = [R3 | I3 | R4 | I4] blocks of (smp, q, tI_l)
                rhs2v = rhs2.rearrange("p (s q blk) -> p s q blk", s=2, q=NQ)
                selA = rhs2v[:, :, :, 0:16]
                selCr = rhs2v[:, :, :, 16:32]
                selCi = rhs2v[:, :, :, 32:48]
                psQ = ps2pool.tile([64, 512], DT, tag="psQ")
                C0 = dftc_sb[:, 0:64]
                S0 = dftc_sb[:, 64:128]
                S0n = dftc_sb[:, 128:192]
                C1 = dftc_sb[:, 192:256]
                S1 = dftc_sb[:, 256:320]
                nc.tensor.matmul(psQ[:, 0:128], C0, selCr, start=True, stop=False)
                nc.tensor.matmul(psQ[:, 0:128], S0, selCi, start=False, stop=True)
                nc.tensor.matmul(psQ[:, 128:256], C0, selCi, start=True, stop=False)
                nc.tensor.matmul(psQ[:, 128:256], S0n, selCr, start=False, stop=True)
                nc.tensor.matmul(psQ[:, 256:384], C1, selA, start=True, stop=True)
                nc.tensor.matmul(psQ[:, 384:512], S1, selA, start=True, stop=True)

                # magnitudes: m = 0.5*sqrt(re^2 + im^2), [64, 128] each;
                # vt = m3 + m4
                Sq = mybir.ActivationFunctionType.Square
                Sqrt = mybir.ActivationFunctionType.Sqrt
                def square(dst, src):
                    if cfg["sq_dve"]:
                        nc.vector.tensor_mul(dst, src, src)
                    else:
                        nc.scalar.activation(dst, src, Sq)

                if cfg["mag_compact"]:
                    # psQ layout (g: tile3/tile4, h: re/im, c): one wide op per step
                    p34 = mpool.tile([64, 512], DT, tag="p34")
                    square(p34[:], psQ[:])
                    p34v = p34.rearrange("p (g h c) -> p g h c", g=2, h=2)
                    s34 = mpool.tile([64, 256], DT, tag="s34")
                    s34v = s34.rearrange("p (g c) -> p g c", g=2)
                    nc.vector.tensor_add(s34v[:], p34v[:, :, 0], p34v[:, :, 1])
                    m34 = mpool.tile([64, 256], DT, tag="m34")
                    nc.scalar.activation(m34[:], s34[:], Sqrt, scale=0.25)
                    m34v = m34.rearrange("p (g c) -> p g c", g=2)
                    vt = mpool.tile([64, 128], dt_w3, tag="vt")
                    nc.vector.tensor_add(vt[:], m34v[:, 0], m34v[:, 1])
                else:
                    p3 = mpool.tile([64, 128], DT, tag="p3")
                    square(p3[:], psQ[:, 0:128])
                    q3 = mpool.tile([64, 128], DT, tag="q3")
                    square(q3[:], psQ[:, 128:256])
                    s3 = mpool.tile([64, 128], DT, tag="s3")
                    nc.vector.tensor_add(s3[:], p3[:], q3[:])
                    m3 = mpool.tile([64, 128], DT, tag="m3")
                    nc.scalar.activation(m3[:], s3[:], Sqrt, scale=0.25)
                    p4 = mpool.tile([64, 128], DT, tag="p4")
                    square(p4[:], psQ[:, 256:384])
                    q4 = mpool.tile([64, 128], DT, tag="q4")
                    square(q4[:], psQ[:, 384:512])
                    s4 = mpool.tile([64, 128], DT, tag="s4")
                    nc.vector.tensor_add(s4[:], p4[:], q4[:])
                    m4 = mpool.tile([64, 128], DT, tag="m4")
                    nc.scalar.activation(m4[:], s4[:], Sqrt, scale=0.25)
                    vt = mpool.tile([64, 128], dt_w3, tag="vt")
                    nc.vector.tensor_add(vt[:], m3[:], m4[:])

                # ---- stage 3: width resize; out partitions = (smp, tI) ----
                # outsm free layout: col 512*p + j holds out row (8*tI + p), col j
                cpat = cfg["copy_pat"]

                def drain(dst, src, i):
                    e = cpat[i % len(cpat)]
                    if e == "v":
                        nc.vector.tensor_copy(dst, src)
                    elif e == "a":
                        nc.scalar.copy(dst, src)
                    else:
                        nc.gpsimd.tensor_copy(dst, src)

                if cfg["wide_ps3"]:
                    for pp in range(4):
                        ps3w = ps3pool.tile([128, 2 * W], DT, tag="ps3w")
                        for half in range(2):
                            nc.tensor.matmul(
                                ps3w[:, W * half : W * half + W],
                                vt[:],
                                wrep_sb[:, W * (2 * pp + half) : W * (2 * pp + half) + W],
                                start=True, stop=True,
                            )
                        drain(outsm[:, 2 * W * pp : 2 * W * pp + 2 * W], ps3w[:], pp)
                else:
                    for p in range(8):
                        ps3 = ps3pool.tile([128, W], DT, tag="ps3")
                        nc.tensor.matmul(
                            ps3[:],
                            vt[:],
                            wrep_sb[:, W * p : W * p + W],
                            start=True, stop=True,
                        )
                        drain(outsm[:, W * p : W * p + W], ps3[:], p)

                _emit_stores(nc, cfg, sqs, ys, bg2, outsm, si)
                si += 2 * cfg["store_parts"]

            if rep_ctx is not None:
                rep_ctx.__exit__(None, None, None)

    nc.compile()
    return nc


_NC = None


def _get_program():
    global _NC
    if _NC is None:
        _NC = _build_program()
    return _NC


def kernel(x: np.ndarray) -> np.ndarray:
    assert x.shape == (B_FULL, 3, H, W), x.shape
    nc = _get_program()
    in_maps = _core_in_maps(x)
    res = run_bass_kernel_spmd(nc, in_maps, core_ids=list(range(N_CORES)))
    out = np.concatenate([res.results[c]["ys"] for c in range(N_CORES)], axis=0)
    return np.ascontiguousarray(out, dtype=np.float32)


# revision 49
# speedup vs baseline: 1.9552x; 1.1325x over previous
"""Trainium2 Bass kernel for nn_DCTHighPass.

Reference computation (per sample, 512x512 RGB image):
  gray = 0.299 R + 0.587 G + 0.114 B
  tiles = 8x8 blocks of gray (64x64 tiles, row-major (ti, tj))
  mag = |fft2(tile)|
  (buggy mask touches only batch 3:6 / fft rows 3:6 -> never sampled below)
  img = mag tiles stacked into [4096*8, 8]
  out = bilinear_resize(img, 512, 512)

Key reduction: the height downsample (32768 -> 512, factor 64) samples only
input rows 64i+31 and 64i+32 with weight 0.5 each, i.e. fft-row 7 of tile
(ti=i//8, tj=8*(i%8)+3) and fft-row 0 of tile (ti, tj=8*(i%8)+4).  fft rows
0/7 of an 8x8 real tile need only three 8-weight row reductions of the tile
(plain sum, cos, sin), followed by an 8-point DFT along columns.  The width
upsample (8 -> 512) is a fixed [8,512] matrix.  So per output row i:
  v = 0.5*(|DFT(cos/sin rowsums of tile tj3)| + |DFT(colsum of tile tj4)|)
  out[i, :] = v @ W8
Only image columns 64p+24 .. 64p+39 (p = i%8) are ever used, so the host
pre-gathers exactly those 128 columns per row into a [B, H, 3, 128]
row-major tensor; every DMA run is then >= 1536B contiguous (full DMA-bus
rate; the original 64B gather runs paid the sub-512B 2x latency penalty).

Pipeline per sample (per core, batch of 8 samples):
  load  (1 DMA):  xs[smp] -> SBUF [128=(row%128), (q, ch, c)=1536]
  gray  (ACT/DVE): 0.299 R + 0.587 G + 0.114 B -> [128, (q c)=512]
  stage1 (PE):  per q, gray chunk stationary, {1,cos,sin}-rowsum weights
                -> PSUM [128=(p,cc), 192=(q,type,tI_l)]
  stage2 (PE):  4 block-diag DFT matmuls -> [64=(p,k), 192] cos/sin x g0/g1
  magnitude (ACT/DVE): sqrt of sum of squares, x0.5 -> V [64=(p,k), 64=tI]
  stage3 (PE):  V (stationary) @ p-masked replicated W8 -> outsm[:, 512p]
  store (1 DMA): outsm sample half [64=(tI), (p,j)=4096] -> ys rows 8*tI+p
"""

import sys

sys.path.insert(0, "/opt/trn_rl_repo")

import numpy as np

from concourse import bacc
import concourse.mybir as mybir
from concourse.tile import TileContext
from concourse.bass_utils import run_bass_kernel_spmd

N_CORES = 8
B_FULL = 64
B_CORE = B_FULL // N_CORES  # 8 samples per core
H = W = 512
K = 8  # fft tile size
NQ = 4  # 128-row chunks per image
NC_COLS = 128  # needed image columns per row (16 of every 64)
DT = mybir.dt.float32

# image columns ever sampled by the width resize: 64p+24 .. 64p+39
COLS = (np.arange(K)[:, None] * 64 + 24 + np.arange(16)[None, :]).reshape(-1)


# ----------------------------------------------------------------------------
# host-side constants
# ----------------------------------------------------------------------------
def _make_constants():
    j = np.arange(K)
    cosr = np.cos(2 * np.pi * j / K)
    sinr = np.sin(2 * np.pi * j / K)

    # wred48 [128, 48]: free = 16*type + tI_l; type 0: plain sum, 1: cos, 2: sin
    # wred144 [128, 144]: same but per-channel gray-coefficient scaled blocks
    wtypes = [np.ones(K), cosr, sinr]
    wred = np.zeros((128, 48), dtype=np.float32)
    for ty in range(3):
        for t in range(16):
            rows = slice(8 * t, 8 * t + 8)
            wred[rows, 16 * ty + t] = wtypes[ty]
    coef = [0.299, 0.587, 0.114]
    wred144 = np.zeros((128, 144), dtype=np.float32)
    for ch in range(3):
        wred144[:, 48 * ch : 48 * ch + 48] = coef[ch] * wred

    # dft matrices C[v,c] = cos(2pi v c/8), S[v,c] = sin(2pi v c/8)
    v = np.arange(K)
    C8 = np.cos(2 * np.pi * np.outer(v, j) / K).astype(np.float32)
    S8 = np.sin(2 * np.pi * np.outer(v, j) / K).astype(np.float32)

    # dftc [128, 320]: 5 blocks of 64 cols: [C*g0 | S*g0 | -S*g0 | C*g1 | S*g1]
    # partition = 16p + cc (cc in 0..15, g = cc//8); out col = 64*s + 8p + k
    dftc = np.zeros((128, 320), dtype=np.float32)
    for p in range(8):
        for cc in range(16):
            g, c = divmod(cc, 8)
            for k in range(8):
                cv, sv = C8[k, c], S8[k, c]
                if g == 0:
                    dftc[16 * p + cc, 0 + 8 * p + k] = cv
                    dftc[16 * p + cc, 64 + 8 * p + k] = sv
                    dftc[16 * p + cc, 128 + 8 * p + k] = -sv
                else:
                    dftc[16 * p + cc, 192 + 8 * p + k] = cv
                    dftc[16 * p + cc, 256 + 8 * p + k] = sv

    # W8 [8, 512]: bilinear width resize 8 -> 512 (align_corners=False)
    src = (np.arange(W) + 0.5) * (K / W) - 0.5
    src = np.clip(src, 0.0, K - 1.0)
    i0 = np.floor(src).astype(np.int64)
    i1 = np.minimum(i0 + 1, K - 1)
    fr = (src - i0).astype(np.float32)
    W8 = np.zeros((K, W), dtype=np.float32)
    for jj in range(W):
        W8[i0[jj], jj] += 1.0 - fr[jj]
        W8[i1[jj], jj] += fr[jj]

    # wrep [64, 8*512]: block p holds W8 on partitions 8p..8p+7, zero elsewhere
    wrep = np.zeros((64, 8 * W), dtype=np.float32)
    for p in range(8):
        wrep[8 * p : 8 * p + 8, W * p : W * p + W] = W8

    return dict(wred48=wred, wred144=wred144, dftc=dftc, wrep=wrep)


_CONSTS = _make_constants()
_DFTC = _CONSTS["dftc"]
_WREP = _CONSTS["wrep"]


def _in_np_dtype(cfg):
    import ml_dtypes

    return ml_dtypes.bfloat16 if cfg["in_bf16"] else np.float32


def _core_in_maps(x: np.ndarray, cfg=None) -> list[dict]:
    cfg = {**CFG, **(cfg or {})}
    xg = np.ascontiguousarray(
        x[:, :, :, COLS].transpose(0, 2, 1, 3), dtype=_in_np_dtype(cfg)
    )
    wred = _CONSTS["wred144"] if cfg["gray_fold"] else _CONSTS["wred48"]
    if cfg["gray_fold"] and cfg["in_bf16"]:
        wred = wred.astype(_in_np_dtype(cfg))
    import ml_dtypes

    wrep = _WREP.astype(ml_dtypes.bfloat16) if cfg["bf3"] else _WREP
    dftc = _DFTC.astype(ml_dtypes.bfloat16) if cfg["bf2"] else _DFTC
    return [
        {
            "xs": xg[c * B_CORE : (c + 1) * B_CORE],
            "wred": wred,
            "dftc": dftc,
            "wrep": wrep,
        }
        for c in range(N_CORES)
    ]


# ----------------------------------------------------------------------------
# bass program (identical on all cores; per-core inputs differ)
# ----------------------------------------------------------------------------
CFG = dict(
    load_pair=False,  # single DMA loads both samples of an iteration
    load_parts=1,   # DMAs per sample for the input load: 1, 3 (per ch), 12 (ch,q)
    store_parts=1,  # DMAs per sample for the output store: 1, 2, 4 (t ranges)
    load_qs="s",    # queue cycle for loads: s=sync, c=scalar, g=gpsimd
    store_qs="s",   # queue cycle for stores (SP queue: no compute to stall)
    in_bf16=True,   # ship x columns as bf16 (halves load traffic)
    out_bf16=True,  # store ys as bf16, upcast on host (halves store traffic)
    gray_fold=True, # fold gray coefficients into stage-1 matmul weights
    mag_compact=False,  # 4-op magnitude chain instead of 9
    wide_ps3=False,     # 2-bank stage-3 PSUM tiles, one copy drains two blocks
    bf3=True,           # stage-3 matmul in bf16 (vt + wrep)
    copy_pat="vava",    # stage-3 drain engines per p (cycled): v=DVE a=Act g=Pool
    xin_bufs=4,
    outp_bufs=2,
    ps_bufs=None,       # (ps1, ps2, ps3) PSUM pool bufs override
    bf2=False,          # stage-2 matmul in bf16 (rhs2 + dftc)
    sq_dve=False,       # magnitude squares on DVE instead of Act
    prefetch=True,      # emit next iteration's loads before current stores
    unroll=2,           # batch-body copies inside the For_i repeat loop
    skip_loads=False,   # timing diagnostics only -- breaks correctness
    skip_stores=False,
    skip_compute=False,
)


def _emit_stores(nc, cfg, sqs, ys, bg2, outsm, si):
    # stores: rows 8*tI + p are 16KB-contiguous per tI; split over t ranges
    # if store_parts > 1
    if cfg["skip_stores"]:
        return
    sp = cfg["store_parts"]
    tchunk = 64 // sp
    for smp in range(2):
        bg = 2 * bg2 + smp
        ydst = ys[bg, 0].rearrange("(t r) j -> t (r j)", r=8)
        for u in range(sp):
            t0 = u * tchunk
            sqs[si % len(sqs)].dma_start(
                ydst[t0 : t0 + tchunk],
                outsm[64 * smp + t0 : 64 * smp + t0 + tchunk, :],
            )
            si += 1


def _build_program(repeat=1, cfg=None):
    cfg = {**CFG, **(cfg or {})}
    nc = bacc.Bacc()

    dt_in = mybir.dt.bfloat16 if cfg["in_bf16"] else DT
    dt_out = mybir.dt.bfloat16 if cfg["out_bf16"] else DT
    wred_shape = [128, 144] if cfg["gray_fold"] else [128, 48]
    dt_wred = dt_in if cfg["gray_fold"] else DT

    dt_w3 = mybir.dt.bfloat16 if cfg["bf3"] else mybir.dt.float32r
    dt_s2 = mybir.dt.bfloat16 if cfg["bf2"] else DT

    xs = nc.declare_dram_parameter("xs", [B_CORE, H, 3, NC_COLS], dt_in, isOutput=False)
    wred_d = nc.declare_dram_parameter("wred", wred_shape, dt_wred, isOutput=False)
    dftc_d = nc.declare_dram_parameter("dftc", [128, 320], dt_s2, isOutput=False)
    wrep_d = nc.declare_dram_parameter("wrep", [64, 8 * W], dt_w3, isOutput=False)
    ys = nc.declare_dram_parameter("ys", [B_CORE, 1, H, W], dt_out, isOutput=True)

    with TileContext(nc) as tc:
        ps1_bufs, ps2_bufs, ps3_bufs = (
            cfg["ps_bufs"]
            if cfg["ps_bufs"]
            else ((2, 2, 2) if cfg["wide_ps3"] else (3, 2, 3))
        )
        with (
            tc.tile_pool(name="consts", bufs=1) as cpool,
            tc.tile_pool(name="xin", bufs=cfg["xin_bufs"]) as xpool,
            tc.tile_pool(name="mid", bufs=2) as mpool,
            tc.tile_pool(name="outp", bufs=cfg["outp_bufs"]) as opool,
            tc.tile_pool(name="ps1", bufs=ps1_bufs, space="PSUM") as ps1pool,
            tc.tile_pool(name="ps2", bufs=ps2_bufs, space="PSUM") as ps2pool,
            tc.tile_pool(name="ps3", bufs=ps3_bufs, space="PSUM") as ps3pool,
        ):
            wred_sb = cpool.tile(wred_shape, dt_wred, tag="wred")
            nc.sync.dma_start(wred_sb[:], wred_d[:])
            dftc_sb = cpool.tile([128, 320], dt_s2, tag="dftc")
            nc.sync.dma_start(dftc_sb[:], dftc_d[:])
            wrep_sb = cpool.tile([64, 8 * W], dt_w3, tag="wrep")
            nc.scalar.dma_start(wrep_sb[:], wrep_d[:])

            queues = dict(s=nc.sync, c=nc.scalar, v=nc.vector, g=nc.gpsimd)
            lqs = [queues[q] for q in cfg["load_qs"]]
            sqs = [queues[q] for q in cfg["store_qs"]]
            li = si = 0

            lcnt = [0]

            def emit_loads(bg2):
                xn = []
                if cfg["load_pair"]:
                    xneed = xpool.tile([128, 2 * NQ * 3 * NC_COLS], dt_in, tag="xnp")
                    xneedv2 = xneed.rearrange(
                        "p (b q ch c) -> p b q ch c", b=2, q=NQ, ch=3
                    )
                    xsrc2 = xs[2 * bg2 : 2 * bg2 + 2].rearrange(
                        "b (q p) ch c -> p b q ch c", p=128
                    )
                    if cfg["skip_loads"]:
                        nc.vector.memset(xneed[:], 0.5)
                    else:
                        lqs[lcnt[0] % len(lqs)].dma_start(xneedv2[:], xsrc2)
                        lcnt[0] += 1
                    return [xneedv2[:, 0], xneedv2[:, 1]]
                for smp in range(2):
                    bg = 2 * bg2 + smp
                    xneed = xpool.tile([128, NQ * 3 * NC_COLS], dt_in, tag=f"xn{smp}")
                    xneedv = xneed.rearrange(
                        "p (q ch c) -> p q ch c", q=NQ, ch=3
                    )
                    xsrc = xs[bg].rearrange("(q p) ch c -> p q ch c", p=128)
                    if cfg["skip_loads"]:
                        nc.vector.memset(xneed[:], 0.5)
                        xn.append(xneedv)
                        continue
                    if cfg["load_parts"] == 1:
                        lqs[lcnt[0] % len(lqs)].dma_start(xneedv[:], xsrc)
                        lcnt[0] += 1
                    elif cfg["load_parts"] == 3:
                        for ch in range(3):
                            lqs[lcnt[0] % len(lqs)].dma_start(
                                xneedv[:, :, ch], xsrc[:, :, ch]
                            )
                            lcnt[0] += 1
                    else:
                        for ch in range(3):
                            for q in range(NQ):
                                lqs[lcnt[0] % len(lqs)].dma_start(
                                    xneedv[:, q, ch], xsrc[:, q, ch]
                                )
                                lcnt[0] += 1
                    xn.append(xneedv)
                return xn

            unroll = cfg["unroll"] if repeat > 1 else 1
            rep_ctx = tc.For_i(0, repeat // unroll, 1) if repeat > 1 else None
            if rep_ctx is not None:
                rep_ctx.__enter__()
            n_it = (B_CORE // 2) * unroll
            preloaded = {}
            for it in range(n_it):
                bg2 = it % (B_CORE // 2)
                # ---- two samples per iteration: stage2/3 run at 128-wide ----
                xn = preloaded.pop(it) if it in preloaded else emit_loads(bg2)

                # ---- gray conversion + stage 1 row reductions per sample ----
                outsm = opool.tile([128, K * W], dt_out, tag="outsm")
                if cfg["skip_compute"]:
                    nc.vector.memset(outsm[:, 0:8], 0.5)
                    _emit_stores(nc, cfg, sqs, ys, bg2, outsm, si)
                    si += 2 * cfg["store_parts"]
                    continue
                rhs2 = mpool.tile([128, 2 * 192], dt_s2, tag="rhs2")
                if cfg["gray_fold"]:
                    # stage-1 matmuls read raw channels; gray coefficients are
                    # folded into per-channel weight blocks (PSUM-accumulated)
                    for smp in range(2):
                        xflat = xn[smp].rearrange("p q ch c -> p (q ch) c")
                        ps1 = ps1pool.tile([128, 192], DT, tag="ps1")
                        for q in range(NQ):
                            for ch in range(3):
                                nc.tensor.matmul(
                                    ps1[:, 48 * q : 48 * q + 48],
                                    xflat[:, 3 * q + ch],
                                    wred_sb[:, 48 * ch : 48 * ch + 48],
                                    start=(ch == 0),
                                    stop=(ch == 2),
                                )
                        nc.vector.tensor_copy(
                            rhs2[:, 192 * smp : 192 * smp + 192], ps1[:]
                        )
                else:
                    xgray = mpool.tile([128, 2 * 512], DT, tag="xgray")
                    for smp in range(2):
                        xg = xgray[:, 512 * smp : 512 * smp + 512]
                        rch = xn[smp][:, :, 0]
                        gch = xn[smp][:, :, 1]
                        bch = xn[smp][:, :, 2]
                        t1 = mpool.tile([128, 512], DT, tag=f"t1{smp}")
                        t1v = t1.rearrange("p (q c) -> p q c", q=NQ)
                        nc.scalar.activation(
                            t1v[:], gch,
                            mybir.ActivationFunctionType.Copy, scale=0.587,
                        )
                        t2 = mpool.tile([128, 512], DT, tag=f"t2{smp}")
                        t2v = t2.rearrange("p (q c) -> p q c", q=NQ)
                        nc.vector.scalar_tensor_tensor(
                            t2v[:], rch, 0.299, t1v[:],
                            mybir.AluOpType.mult, mybir.AluOpType.add,
                        )
                        xgv4 = xg.rearrange("p (q c) -> p q c", q=NQ)
                        nc.vector.scalar_tensor_tensor(
                            xgv4[:], bch, 0.114, t2v[:],
                            mybir.AluOpType.mult, mybir.AluOpType.add,
                        )
                    xgv = xgray.rearrange("p (sq c) -> p sq c", c=128)
                    for smp in range(2):
                        ps1 = ps1pool.tile([128, 192], DT, tag="ps1")
                        for q in range(NQ):
                            nc.tensor.matmul(
                                ps1[:, 48 * q : 48 * q + 48],
                                xgv[:, 4 * smp + q],
                                wred_sb[:],
                                start=True, stop=True,
                            )
                        nc.vector.tensor_copy(
                            rhs2[:, 192 * smp : 192 * smp + 192], ps1[:]
                        )

                # ---- stage 2: DFT + height-blend fused via PSUM accumulation
                # psQ [64=(p,k), 512] = [R3 | I3 | R4 | I4] blocks of (smp, q, tI_l)
                rhs2v = rhs2.rearrange("p (s q blk) -> p s q blk", s=2, q=NQ)
                selA = rhs2v[:, :, :, 0:16]
                selCr = rhs2v[:, :, :, 16:32]
                selCi = rhs2v[:, :, :, 32:48]
                psQ = ps2pool.tile([64, 512], DT, tag="psQ")
                C0 = dftc_sb[:, 0:64]
                S0 = dftc_sb[:, 64:128]
                S0n = dftc_sb[:, 128:192]
                C1 = dftc_sb[:, 192:256]
                S1 = dftc_sb[:, 256:320]
                nc.tensor.matmul(psQ[:, 0:128], C0, selCr, start=True, stop=False)
                nc.tensor.matmul(psQ[:, 0:128], S0, selCi, start=False, stop=True)
                nc.tensor.matmul(psQ[:, 128:256], C0, selCi, start=True, stop=False)
                nc.tensor.matmul(psQ[:, 128:256], S0n, selCr, start=False, stop=True)
                nc.tensor.matmul(psQ[:, 256:384], C1, selA, start=True, stop=True)
                nc.tensor.matmul(psQ[:, 384:512], S1, selA, start=True, stop=True)

                # magnitudes: m = 0.5*sqrt(re^2 + im^2), [64, 128] each;
                # vt = m3 + m4
                Sq = mybir.ActivationFunctionType.Square
                Sqrt = mybir.ActivationFunctionType.Sqrt
                def square(dst, src):
                    if cfg["sq_dve"]:
                        nc.vector.tensor_mul(dst, src, src)
                    else:
                        nc.scalar.activation(dst, src, Sq)

                if cfg["mag_compact"]:
                    # psQ layout (g: tile3/tile4, h: re/im, c): one wide op per step
                    p34 = mpool.tile([64, 512], DT, tag="p34")
                    square(p34[:], psQ[:])
                    p34v = p34.rearrange("p (g h c) -> p g h c", g=2, h=2)
                    s34 = mpool.tile([64, 256], DT, tag="s34")
                    s34v = s34.rearrange("p (g c) -> p g c", g=2)
                    nc.vector.tensor_add(s34v[:], p34v[:, :, 0], p34v[:, :, 1])
                    m34 = mpool.tile([64, 256], DT, tag="m34")
                    nc.scalar.activation(m34[:], s34[:], Sqrt, scale=0.25)
                    m34v = m34.rearrange("p (g c) -> p g c", g=2)
                    vt = mpool.tile([64, 128], dt_w3, tag="vt")
                    nc.vector.tensor_add(vt[:], m34v[:, 0], m34v[:, 1])
                else:
                    p3 = mpool.tile([64, 128], DT, tag="p3")
                    square(p3[:], psQ[:, 0:128])
                    q3 = mpool.tile([64, 128], DT, tag="q3")
                    square(q3[:], psQ[:, 128:256])
                    s3 = mpool.tile([64, 128], DT, tag="s3")
                    nc.vector.tensor_add(s3[:], p3[:], q3[:])
                    m3 = mpool.tile([64, 128], DT, tag="m3")
                    nc.scalar.activation(m3[:], s3[:], Sqrt, scale=0.25)
                    p4 = mpool.tile([64, 128], DT, tag="p4")
                    square(p4[:], psQ[:, 256:384])
                    q4 = mpool.tile([64, 128], DT, tag="q4")
                    square(q4[:], psQ[:, 384:512])
                    s4 = mpool.tile([64, 128], DT, tag="s4")
                    nc.vector.tensor_add(s4[:], p4[:], q4[:])
                    m4 = mpool.tile([64, 128], DT, tag="m4")
                    nc.scalar.activation(m4[:], s4[:], Sqrt, scale=0.25)
                    vt = mpool.tile([64, 128], dt_w3, tag="vt")
                    nc.vector.tensor_add(vt[:], m3[:], m4[:])

                # ---- stage 3: width resize; out partitions = (smp, tI) ----
                # outsm free layout: col 512*p + j holds out row (8*tI + p), col j
                cpat = cfg["copy_pat"]

                def drain(dst, src, i):
                    e = cpat[i % len(cpat)]
                    if e == "v":
                        nc.vector.tensor_copy(dst, src)
                    elif e == "a":
                        nc.scalar.copy(dst, src)
                    else:
                        nc.gpsimd.tensor_copy(dst, src)

                if cfg["wide_ps3"]:
                    for pp in range(4):
                        ps3w = ps3pool.tile([128, 2 * W], DT, tag="ps3w")
                        for half in range(2):
                            nc.tensor.matmul(
                                ps3w[:, W * half : W * half + W],
                                vt[:],
                                wrep_sb[:, W * (2 * pp + half) : W * (2 * pp + half) + W],
                                start=True, stop=True,
                            )
                        drain(outsm[:, 2 * W * pp : 2 * W * pp + 2 * W], ps3w[:], pp)
                else:
                    for p in range(8):
                        ps3 = ps3pool.tile([128, W], DT, tag="ps3")
                        nc.tensor.matmul(
                            ps3[:],
                            vt[:],
                            wrep_sb[:, W * p : W * p + W],
                            start=True, stop=True,
                        )
                        drain(outsm[:, W * p : W * p + W], ps3[:], p)

                # prefetch next iteration's loads ahead of this one's stores
                # so loads never queue behind store semaphore waits on SP
                if cfg["prefetch"] and it + 1 < n_it:
                    preloaded[it + 1] = emit_loads((it + 1) % (B_CORE // 2))
                _emit_stores(nc, cfg, sqs, ys, bg2, outsm, si)
                si += 2 * cfg["store_parts"]

            if rep_ctx is not None:
                rep_ctx.__exit__(None, None, None)

    nc.compile()
    return nc


_NC = None


def _get_program():
    global _NC
    if _NC is None:
        _NC = _build_program()
    return _NC


def kernel(x: np.ndarray) -> np.ndarray:
    assert x.shape == (B_FULL, 3, H, W), x.shape
    nc = _get_program()
    in_maps = _core_in_maps(x)
    res = run_bass_kernel_spmd(nc, in_maps, core_ids=list(range(N_CORES)))
    out = np.concatenate([res.results[c]["ys"] for c in range(N_CORES)], axis=0)
    return np.ascontiguousarray(out, dtype=np.float32)
